# revision 2
# baseline (speedup 1.0000x reference)
"""Trainium2 Bass kernel for nn_Decoder_21784074125982 (6-layer TTS-style
transformer decoder), data-parallel over batch across 8 NeuronCores.

Layout: activations channel-major (features on partitions, tokens on free dim)
so every matmul contracts over partitions without transposes.  LayerNorm stats
via ones-matrix matmuls on TensorE (every output row = the cross-partition sum
-> free broadcast).  Attention scores produced pre-transposed (keys on
partitions); softmax = plain exp (logits bounded, no max-subtraction) with the
key-validity mask folded into the exp bias and the causal mask applied as a
0/1 multiply; denominator from a ones-matmul over the exp'd scores.  All
matmuls run in float32r (~13-bit mantissa, full PE rate at N>=256).
"""
import numpy as np

import jax
from jax.sharding import Mesh, PartitionSpec, NamedSharding
from jax.experimental.shard_map import shard_map

import concourse.bass as bass
import concourse.mybir as mybir
import concourse.tile as tile
from concourse import bacc, bass2jax
from contextlib import ExitStack

B, T, DM, H, L, D, RF = 16, 512, 1024, 8, 6, 80, 2
DIN = 4 * DM
PRE_INNER = 256
NCORES = 8
BL = B // NCORES
NKC = DM // 128
NFC = DIN // 128
NT = T // 128
NEG = -1.0e30

F32R = mybir.dt.float32r
F32 = mybir.dt.float32
AF = mybir.ActivationFunctionType
ALU = mybir.AluOpType


# ---------------------------------------------------------------- bass build
def _build(n_layers=L, debug_taps=False):
    nc = bacc.Bacc(trn_type="TRN2", target_bir_lowering=False, debug=False,
                   name="decoder")

    d_tgt = nc.declare_dram_parameter("tgtT", [BL, 2, 128, T], F32R, isOutput=False)
    d_enc = nc.declare_dram_parameter("encT", [BL, NKC, 128, T], F32R, isOutput=False)
    d_kvb = nc.declare_dram_parameter("kvbias", [BL, 128, NT], F32, isOutput=False)

    d_pw1 = nc.declare_dram_parameter("pw1", [2, 2, 128, 128], F32R, isOutput=False)
    d_pw2 = nc.declare_dram_parameter("pw2", [2, NKC, 128, 128], F32R, isOutput=False)
    d_pb = nc.declare_dram_parameter("pb", [128, 2 + NKC], F32, isOutput=False)
    d_ln0 = nc.declare_dram_parameter("ln0", [128, 2 * NKC], F32, isOutput=False)

    d_wq = nc.declare_dram_parameter("wq", [n_layers, NKC, NKC, 128, 128], F32R, isOutput=False)
    d_wk = nc.declare_dram_parameter("wk", [n_layers, NKC, NKC, 128, 128], F32R, isOutput=False)
    d_wv = nc.declare_dram_parameter("wv", [n_layers, NKC, 2, 128, 512], F32R, isOutput=False)
    d_wsfc = nc.declare_dram_parameter("wsfc", [n_layers, NKC, NKC, 128, 128], F32R, isOutput=False)
    d_wcq = nc.declare_dram_parameter("wcq", [n_layers, NKC, NKC, 128, 128], F32R, isOutput=False)
    d_wck = nc.declare_dram_parameter("wck", [n_layers, NKC, NKC, 128, 128], F32R, isOutput=False)
    d_wcv = nc.declare_dram_parameter("wcv", [n_layers, NKC, 2, 128, 512], F32R, isOutput=False)
    d_wcfc = nc.declare_dram_parameter("wcfc", [n_layers, NKC, NKC, 128, 128], F32R, isOutput=False)
    d_w1 = nc.declare_dram_parameter("w1", [n_layers, NKC, NFC, 128, 128], F32R, isOutput=False)
    d_w2 = nc.declare_dram_parameter("w2", [n_layers, NFC, NKC, 128, 128], F32R, isOutput=False)
    d_b1 = nc.declare_dram_parameter("b1", [n_layers, 128, NFC], F32, isOutput=False)
    d_b2 = nc.declare_dram_parameter("b2", [n_layers, 128, NKC], F32, isOutput=False)
    d_lnp = nc.declare_dram_parameter("lnp", [n_layers, 128, 6 * NKC], F32, isOutput=False)
    d_wout = nc.declare_dram_parameter("wout", [NKC, 2, 128, 128], F32R, isOutput=False)
    d_bout = nc.declare_dram_parameter("bout", [128, 2], F32, isOutput=False)
    d_pos = nc.declare_dram_parameter("posT", [NKC, 128, T], F32, isOutput=False)
    d_caus = nc.declare_dram_parameter("causal", [4, 128, 256], F32R, isOutput=False)
    d_ones = nc.declare_dram_parameter("onesm", [128, 128], F32R, isOutput=False)
    d_eps = nc.declare_dram_parameter("epsv", [128, 1], F32, isOutput=False)

    d_out = nc.declare_dram_parameter("outT", [BL, D, T * RF], F32, isOutput=True)
    d_taps = None
    if debug_taps:
        d_taps = nc.declare_dram_parameter(
            "taps", [n_layers + 1, BL, NKC, 128, T], F32, isOutput=True)

    with tile.TileContext(nc) as tc, ExitStack() as ctx:
        hp = ctx.enter_context(tc.tile_pool(name="hp", bufs=1))
        wp = ctx.enter_context(tc.tile_pool(name="wp", bufs=4))
        qp = ctx.enter_context(tc.tile_pool(name="qp", bufs=1))
        kp = ctx.enter_context(tc.tile_pool(name="kp", bufs=1))
        vp = ctx.enter_context(tc.tile_pool(name="vp", bufs=1))
        op = ctx.enter_context(tc.tile_pool(name="op", bufs=1))
        fp = ctx.enter_context(tc.tile_pool(name="fp", bufs=1))
        ap = ctx.enter_context(tc.tile_pool(name="ap", bufs=2))
        lp = ctx.enter_context(tc.tile_pool(name="lp", bufs=1))
        rp = ctx.enter_context(tc.tile_pool(name="rp", bufs=2))
        cp = ctx.enter_context(tc.tile_pool(name="cp", bufs=1))
        bp = ctx.enter_context(tc.tile_pool(name="bp", bufs=2))
        ps = ctx.enter_context(tc.tile_pool(name="ps", bufs=1, space="PSUM"))

        def psum(tag, bufs=None):
            kw = {"bufs": bufs} if bufs else {}
            return ps.tile([128, 512], F32, tag=tag, name=f"ps_{tag}", **kw)

        ones = cp.tile([128, 128], F32R, name="ones")
        nc.sync.dma_start(out=ones[:], in_=d_ones.ap())
        epsv = cp.tile([128, 1], F32, name="epsv")
        nc.sync.dma_start(out=epsv[:], in_=d_eps.ap())
        caus = [cp.tile([128, 256], F32R, name=f"caus{i}") for i in range(4)]
        for i in range(4):
            nc.sync.dma_start(out=caus[i][:], in_=d_caus.ap()[i])
        kvb = [cp.tile([128, NT], F32, name=f"kvb{b}") for b in range(BL)]
        for b in range(BL):
            nc.sync.dma_start(out=kvb[b][:], in_=d_kvb.ap()[b])

        h = [[None] * NKC for _ in range(BL)]

        def mm_stage(w_ap, rhs_tiles, n_oc, evict, n_kc=NKC):
            """out[oc] = sum_kc w[kc,oc].T @ rhs[kc]; evict(oc, psum_ap)."""
            for oc in range(n_oc):
                acc = psum(("m0", "m1")[oc % 2])
                for kc in range(n_kc):
                    wt = wp.tile([128, 128], F32R, tag=f"w{kc}", name=f"w{kc}_{oc}")
                    nc.sync.dma_start(out=wt[:], in_=w_ap[kc, oc])
                    nc.tensor.matmul(acc[:, :T], lhsT=wt[:], rhs=rhs_tiles[kc][:],
                                     start=(kc == 0), stop=(kc == n_kc - 1))
                evict(oc, acc)

        def ln(b, x1, lnp_ap, g_off, b_off):
            """LN over channels; writes fresh h[b] tiles.
            lnp_ap: (128, cols) DRAM AP; g/b at columns g_off+oc / b_off+oc."""
            gt = bp.tile([128, NKC], F32, tag="lng", name="lng")
            nc.sync.dma_start(out=gt[:], in_=lnp_ap[:, g_off:g_off + NKC])
            bt = bp.tile([128, NKC], F32, tag="lnb", name="lnb")
            nc.sync.dma_start(out=bt[:], in_=lnp_ap[:, b_off:b_off + NKC])
            s1 = psum("d")
            for oc in range(NKC):
                nc.tensor.matmul(s1[:, :T], lhsT=ones[:], rhs=x1[oc][:],
                                 start=(oc == 0), stop=(oc == NKC - 1))
            s2 = psum("o")
            for oc in range(NKC):
                sq = ap.tile([128, T], F32R, tag="a0", name=f"sq{oc}")
                nc.scalar.square(out=sq[:], in_=x1[oc][:])
                nc.tensor.matmul(s2[:, :T], lhsT=ones[:], rhs=sq[:],
                                 start=(oc == 0), stop=(oc == NKC - 1))
            m = lp.tile([128, T], F32, tag="ln_m", name="ln_m")
            nc.scalar.mul(out=m[:], in_=s1[:, :T], mul=1.0 / DM)
            t2 = lp.tile([128, T], F32, tag="ln_t2", name="ln_t2")
            nc.scalar.mul(out=t2[:], in_=s2[:, :T], mul=1.0 / DM)
            mm_ = lp.tile([128, T], F32, tag="ln_mm", name="ln_mm")
            nc.vector.tensor_mul(out=mm_[:], in0=m[:], in1=m[:])
            var = lp.tile([128, T], F32, tag="ln_var", name="ln_var")
            nc.vector.tensor_sub(out=var[:], in0=t2[:], in1=mm_[:])
            std = lp.tile([128, T], F32, tag="ln_std", name="ln_std")
            nc.scalar.activation(out=std[:], in_=var[:], func=AF.Sqrt, bias=epsv[:])
            r = lp.tile([128, T], F32, tag="ln_r", name="ln_r")
            nc.vector.reciprocal(out=r[:], in_=std[:])
            mr = lp.tile([128, T], F32, tag="ln_mr", name="ln_mr")
            nc.vector.tensor_mul(out=mr[:], in0=m[:], in1=r[:])
            for oc in range(NKC):
                z = ap.tile([128, T], F32, tag="a1", name=f"z{oc}")
                nc.vector.tensor_mul(out=z[:], in0=x1[oc][:], in1=r[:])
                z2 = ap.tile([128, T], F32, tag="a2", name=f"z2{oc}")
                nc.vector.tensor_sub(out=z2[:], in0=z[:], in1=mr[:])
                hn = hp.tile([128, T], F32R, tag=f"h{b}_{oc}", name=f"h{b}_{oc}")
                nc.scalar.activation(out=hn[:], in_=z2[:], func=AF.Identity,
                                     bias=bt[:, oc:oc + 1], scale=gt[:, oc:oc + 1])
                h[b][oc] = hn

        def choiceA_stage(w_ap, x_tiles, n_kc=NKC):
            """token-major products V[tt][nn] (128t, 512f); w_ap[kc,nn]->(128,512)."""
            outs = [[None, None] for _ in range(NT)]
            for nn in range(2):
                wv_t = []
                for kc in range(n_kc):
                    wt = fp.tile([128, 512], F32R, tag=f"f{8 + kc}", name=f"wv{kc}_{nn}")
                    nc.sync.dma_start(out=wt[:], in_=w_ap[kc, nn])
                    wv_t.append(wt)
                for tt in range(NT):
                    acc = psum(("m0", "m1")[tt % 2])
                    for kc in range(n_kc):
                        nc.tensor.matmul(
                            acc[:],
                            lhsT=x_tiles[kc][:, tt * 128:(tt + 1) * 128],
                            rhs=wv_t[kc][:],
                            start=(kc == 0), stop=(kc == n_kc - 1))
                    vt = vp.tile([128, 512], F32R, tag=f"v{tt}_{nn}",
                                 name=f"v{tt}_{nn}")
                    nc.vector.tensor_copy(out=vt[:], in_=acc[:])
                    outs[tt][nn] = vt
            return outs

        def attention(b, QT, KT, V, out_pool, out_tag, causal):
            outs = []
            for hh in range(H):
                oth = out_pool.tile([128, T], F32R, tag=out_tag(hh),
                                    name=f"att{b}_{hh}")
                qhs = [(0, T)] if not causal else [(0, 256), (256, 256)]
                for qi, (q0, qn) in enumerate(qhs):
                    kcs = list(range(NT) if not causal else range(2 * (qi + 1)))
                    a_tiles = {}
                    for kc in kcs:
                        s_ps = psum("s", bufs=2)
                        nc.tensor.matmul(
                            s_ps[:, :qn],
                            lhsT=KT[hh][:, kc * 128:(kc + 1) * 128],
                            rhs=QT[hh][:, q0:q0 + qn], start=True, stop=True)
                        at = ap.tile([128, 512], F32R, tag=f"a{kc}", name=f"at{kc}")
                        if causal and (kc * 128 + 127 > q0):
                            ae = ap.tile([128, 512], F32R, tag="ae", name=f"ae{kc}")
                            nc.scalar.activation(out=ae[:, :qn], in_=s_ps[:, :qn],
                                                 func=AF.Exp,
                                                 bias=kvb[b][:, kc:kc + 1])
                            nc.vector.tensor_mul(out=at[:, :qn], in0=ae[:, :qn],
                                                 in1=caus[kc][:, :qn])
                        else:
                            nc.scalar.activation(out=at[:, :qn], in_=s_ps[:, :qn],
                                                 func=AF.Exp,
                                                 bias=kvb[b][:, kc:kc + 1])
                        a_tiles[kc] = at
                    d_ps = psum("d")
                    for i, kc in enumerate(kcs):
                        nc.tensor.matmul(d_ps[:, :qn], lhsT=ones[:],
                                         rhs=a_tiles[kc][:, :qn],
                                         start=(i == 0), stop=(i == len(kcs) - 1))
                    rec = rp.tile([128, T], F32, tag="rec", name="rec")
                    nc.vector.reciprocal(out=rec[:, :qn], in_=d_ps[:, :qn])
                    o_ps = psum("o")
                    for i, kc in enumerate(kcs):
                        nc.tensor.matmul(
                            o_ps[:, :qn],
                            lhsT=V[kc][hh // 4][:, (hh % 4) * 128:(hh % 4 + 1) * 128],
                            rhs=a_tiles[kc][:, :qn],
                            start=(i == 0), stop=(i == len(kcs) - 1))
                    nc.vector.tensor_mul(out=oth[:, q0:q0 + qn], in0=o_ps[:, :qn],
                                         in1=rec[:, :qn])
                outs.append(oth)
            return outs

        # ---------------- prenet + ln0 ----------------
        for b in range(BL):
            tgt = []
            for kc in range(2):
                tg = ap.tile([128, T], F32R, tag=f"a{kc}", name=f"tgt{kc}")
                nc.sync.dma_start(out=tg[:], in_=d_tgt.ap()[b, kc])
                tgt.append(tg)
            p1 = []
            for oc in range(2):
                acc = psum(("m0", "m1")[oc % 2])
                for kc in range(2):
                    wt = wp.tile([128, 128], F32R, tag=f"w{kc}", name=f"pw1_{kc}")
                    nc.sync.dma_start(out=wt[:], in_=d_pw1.ap()[kc, oc])
                    nc.tensor.matmul(acc[:, :T], lhsT=wt[:], rhs=tgt[kc][:],
                                     start=(kc == 0), stop=(kc == 1))
                pbv = bp.tile([128, 1], F32, tag="pb", name=f"pb1{oc}")
                nc.sync.dma_start(out=pbv[:], in_=d_pb.ap()[:, oc:oc + 1])
                pt = ap.tile([128, T], F32R, tag=f"a{2 + oc}", name=f"p1_{oc}")
                nc.scalar.activation(out=pt[:], in_=acc[:, :T], func=AF.Relu,
                                     bias=pbv[:])
                p1.append(pt)
            x1 = []
            for oc in range(NKC):
                acc = psum(("m0", "m1")[oc % 2])
                for kc in range(2):
                    wt = wp.tile([128, 128], F32R, tag=f"w{kc}", name=f"pw2_{kc}")
                    nc.sync.dma_start(out=wt[:], in_=d_pw2.ap()[kc, oc])
                    nc.tensor.matmul(acc[:, :T], lhsT=wt[:], rhs=p1[kc][:],
                                     start=(kc == 0), stop=(kc == 1))
                pbv = bp.tile([128, 1], F32, tag="pb", name=f"pb2{oc}")
                nc.sync.dma_start(out=pbv[:], in_=d_pb.ap()[:, 2 + oc:3 + oc])
                rl = ap.tile([128, T], F32, tag="a0", name=f"rl{oc}")
                nc.scalar.activation(out=rl[:], in_=acc[:, :T], func=AF.Relu,
                                     bias=pbv[:])
                pe = ap.tile([128, T], F32, tag="a1", name=f"pe{oc}")
                nc.sync.dma_start(out=pe[:], in_=d_pos.ap()[oc])
                xt = kp.tile([128, T], F32R, tag=f"k{oc}", name=f"x0_{oc}")
                nc.vector.tensor_add(out=xt[:], in0=rl[:], in1=pe[:])
                x1.append(xt)
            ln(b, x1, d_ln0.ap(), 0, NKC)
            if debug_taps:
                for oc in range(NKC):
                    dt_ = ap.tile([128, T], F32, tag="a3", name=f"tp{oc}")
                    nc.vector.tensor_copy(out=dt_[:], in_=h[b][oc][:])
                    nc.sync.dma_start(out=d_taps.ap()[0, b, oc], in_=dt_[:])

        # ---------------- layers ----------------
        for l in range(n_layers):
            for b in range(BL):
                QT, KT = [], []

                def ev_q(oc, acc):
                    t = qp.tile([128, T], F32R, tag=f"q{oc}", name=f"qt{oc}")
                    nc.scalar.copy(out=t[:], in_=acc[:, :T])
                    QT.append(t)

                def ev_k(oc, acc):
                    t = kp.tile([128, T], F32R, tag=f"k{oc}", name=f"kt{oc}")
                    nc.scalar.copy(out=t[:], in_=acc[:, :T])
                    KT.append(t)

                mm_stage(d_wq.ap()[l], h[b], NKC, ev_q)
                mm_stage(d_wk.ap()[l], h[b], NKC, ev_k)
                V = choiceA_stage(d_wv.ap()[l], h[b])

                OT = attention(b, QT, KT, V, op, lambda i: f"o{i}", causal=True)

                x1 = []

                def ev_fc(oc, acc, _b=b):
                    t = kp.tile([128, T], F32R, tag=f"k{oc}", name=f"x1_{oc}")
                    nc.vector.tensor_add(out=t[:], in0=acc[:, :T], in1=h[_b][oc][:])
                    x1.append(t)

                mm_stage(d_wsfc.ap()[l], OT, NKC, ev_fc)
                ln(b, x1, d_lnp.ap()[l], 0, NKC)

                QcT = []

                def ev_cq(oc, acc):
                    t = op.tile([128, T], F32R, tag=f"o{oc}", name=f"qct{oc}")
                    nc.scalar.copy(out=t[:], in_=acc[:, :T])
                    QcT.append(t)

                mm_stage(d_wcq.ap()[l], h[b], NKC, ev_cq)

                enc = []
                for kc in range(NKC):
                    et = fp.tile([128, T], F32R, tag=f"f{kc}", name=f"enc{kc}")
                    nc.sync.dma_start(out=et[:], in_=d_enc.ap()[b, kc])
                    enc.append(et)
                CK = []

                def ev_ck(oc, acc):
                    t = kp.tile([128, T], F32R, tag=f"k{oc}", name=f"ck{oc}")
                    nc.scalar.copy(out=t[:], in_=acc[:, :T])
                    CK.append(t)

                mm_stage(d_wck.ap()[l], enc, NKC, ev_ck)
                CV = choiceA_stage(d_wcv.ap()[l], enc)

                OcT = attention(b, QcT, CK, CV, qp, lambda i: f"q{i}", causal=False)

                x1 = []

                def ev_cfc(oc, acc, _b=b):
                    t = kp.tile([128, T], F32R, tag=f"k{oc}", name=f"cx1_{oc}")
                    nc.vector.tensor_add(out=t[:], in0=acc[:, :T], in1=h[_b][oc][:])
                    x1.append(t)

                mm_stage(d_wcfc.ap()[l], OcT, NKC, ev_cfc)
                ln(b, x1, d_lnp.ap()[l], 2 * NKC, 3 * NKC)

                # --- ffn in two DIN halves ---
                partial = []
                x1 = []
                for half in range(2):
                    F_t = []
                    for fc in range(NFC // 2):
                        oc = half * (NFC // 2) + fc
                        acc = psum(("m0", "m1")[fc % 2])
                        for kc in range(NKC):
                            wt = wp.tile([128, 128], F32R, tag=f"w{kc}",
                                         name=f"w1_{kc}_{oc}")
                            nc.sync.dma_start(out=wt[:], in_=d_w1.ap()[l, kc, oc])
                            nc.tensor.matmul(acc[:, :T], lhsT=wt[:], rhs=h[b][kc][:],
                                             start=(kc == 0), stop=(kc == NKC - 1))
                        bv = bp.tile([128, 1], F32, tag="b1", name=f"b1_{oc}")
                        nc.sync.dma_start(out=bv[:], in_=d_b1.ap()[l][:, oc:oc + 1])
                        ft = fp.tile([128, T], F32R, tag=f"f{fc}", name=f"ff{oc}")
                        nc.scalar.activation(out=ft[:], in_=acc[:, :T], func=AF.Relu,
                                             bias=bv[:])
                        F_t.append(ft)
                    for oc in range(NKC):
                        acc = psum(("m0", "m1")[oc % 2])
                        for fc in range(NFC // 2):
                            kc = half * (NFC // 2) + fc
                            wt = wp.tile([128, 128], F32R, tag=f"w{fc % NKC}",
                                         name=f"w2_{kc}_{oc}")
                            nc.sync.dma_start(out=wt[:], in_=d_w2.ap()[l, kc, oc])
                            nc.tensor.matmul(acc[:, :T], lhsT=wt[:], rhs=F_t[fc][:],
                                             start=(fc == 0),
                                             stop=(fc == NFC // 2 - 1))
                        if half == 0:
                            pt = op.tile([128, T], F32, tag=f"o{oc}", name=f"pp{oc}")
                            nc.vector.tensor_add(out=pt[:], in0=acc[:, :T],
                                                 in1=h[b][oc][:])
                            partial.append(pt)
                        else:
                            bv = bp.tile([128, 1], F32, tag="b2", name=f"b2_{oc}")
                            nc.sync.dma_start(out=bv[:],
                                              in_=d_b2.ap()[l][:, oc:oc + 1])
                            t = kp.tile([128, T], F32R, tag=f"k{oc}",
                                        name=f"fx1_{oc}")
                            nc.vector.scalar_tensor_tensor(
                                out=t[:], in0=acc[:, :T], scalar=bv[:],
                                in1=partial[oc][:], op0=ALU.add, op1=ALU.add)
                            x1.append(t)
                ln(b, x1, d_lnp.ap()[l], 4 * NKC, 5 * NKC)

                if debug_taps:
                    for oc in range(NKC):
                        dt_ = ap.tile([128, T], F32, tag="a3", name=f"tp{oc}")
                        nc.vector.tensor_copy(out=dt_[:], in_=h[b][oc][:])
                        nc.sync.dma_start(out=d_taps.ap()[l + 1, b, oc], in_=dt_[:])

        # ---------------- output projection ----------------
        for b in range(BL):
            o_view = d_out.ap()[b].rearrange("d (t two) -> d t two", two=2)
            for oc in range(2):
                acc = psum(("m0", "m1")[oc % 2])
                for kc in range(NKC):
                    wt = wp.tile([128, 128], F32R, tag=f"w{kc}", name=f"wo{kc}_{oc}")
                    nc.sync.dma_start(out=wt[:], in_=d_wout.ap()[kc, oc])
                    nc.tensor.matmul(acc[:, :T], lhsT=wt[:], rhs=h[b][kc][:],
                                     start=(kc == 0), stop=(kc == NKC - 1))
                bv = bp.tile([128, 1], F32, tag="bo", name=f"bo{oc}")
                nc.sync.dma_start(out=bv[:], in_=d_bout.ap()[:, oc:oc + 1])
                oo = ap.tile([128, T], F32, tag=f"a{oc}", name=f"oo{oc}")
                nc.scalar.activation(out=oo[:], in_=acc[:, :T], func=AF.Identity,
                                     bias=bv[:])
                nc.sync.dma_start(out=o_view[:, :, oc], in_=oo[:D, :])

    nc.finalize()
    return nc


# ------------------------------------------------------------- host prep
def _posenc_np(n_t, n_c):
    depth = (np.arange(n_c) // 2 * 2).astype(np.float64)
    denom = np.power(10000.0, depth / n_c)
    phase = np.arange(n_t)[:, None] / denom[None]
    phase[:, ::2] += np.pi / 2
    return np.sin(phase).astype(np.float32)  # (T, C)


def _chunkB(w):
    """(out, in) weight -> lhsT tiles (kc, oc, 128, 128) of W^T."""
    o, i = w.shape
    return np.ascontiguousarray(
        w.T.reshape(i // 128, 128, o // 128, 128).transpose(0, 2, 1, 3))


def _chunkA(w):
    """(out, in) weight -> rhs tiles (kc, nn, 128, 512) of W^T."""
    o, i = w.shape
    return np.ascontiguousarray(
        w.T.reshape(i // 128, 128, o // 512, 512).transpose(0, 2, 1, 3))


def _pp(v):
    """per-partition param (n*128,) -> (128, n)"""
    v = np.asarray(v, np.float32)
    return np.ascontiguousarray(v.reshape(-1, 128).T)


def _prep_inputs(enc_output, target, data_len, params, n_layers=L):
    p = {k: np.asarray(v, dtype=np.float32) for k, v in params.items()}
    enc_output = np.asarray(enc_output, np.float32)
    target = np.asarray(target, np.float32)
    data_len = np.asarray(data_len).astype(np.int64)

    sh = {}
    Wqkv = p["self_Wqkv"]
    sh["wq"] = np.stack([_chunkB(Wqkv[l, 0:DM]) for l in range(n_layers)])
    sh["wk"] = np.stack([_chunkB(Wqkv[l, DM:2 * DM]) for l in range(n_layers)])
    sh["wv"] = np.stack([_chunkA(Wqkv[l, 2 * DM:3 * DM]) for l in range(n_layers)])
    sh["wsfc"] = np.stack([_chunkB(p["self_Wfc"][l]) for l in range(n_layers)])
    sh["wcq"] = np.stack([_chunkB(p["cross_Wq"][l]) for l in range(n_layers)])
    Wkv = p["cross_Wkv"]
    sh["wck"] = np.stack([_chunkB(Wkv[l, 0:DM]) for l in range(n_layers)])
    sh["wcv"] = np.stack([_chunkA(Wkv[l, DM:2 * DM]) for l in range(n_layers)])
    sh["wcfc"] = np.stack([_chunkB(p["cross_Wfc"][l]) for l in range(n_layers)])
    sh["w1"] = np.stack([_chunkB(p["ffn_w1"][l]) for l in range(n_layers)])
    sh["w2"] = np.stack([_chunkB(p["ffn_w2"][l]) for l in range(n_layers)])
    sh["b1"] = np.stack([_pp(p["ffn_b1"][l]) for l in range(n_layers)])
    sh["b2"] = np.stack([_pp(p["ffn_b2"][l]) for l in range(n_layers)])
    sh["lnp"] = np.stack([
        np.concatenate([_pp(p["self_ln_g"][l]), _pp(p["self_ln_b"][l]),
                        _pp(p["cross_ln_g"][l]), _pp(p["cross_ln_b"][l]),
                        _pp(p["ffn_ln_g"][l]), _pp(p["ffn_ln_b"][l])], axis=1)
        for l in range(n_layers)])

    pw1 = np.zeros((PRE_INNER, 256), np.float32)
    pw1[:, :D * RF] = p["prenet_w1"]
    sh["pw1"] = _chunkB(pw1)
    sh["pw2"] = _chunkB(p["prenet_w2"])
    sh["pb"] = np.concatenate([_pp(p["prenet_b1"]), _pp(p["prenet_b2"])], axis=1)
    sh["ln0"] = np.concatenate([_pp(p["ln0_g"]), _pp(p["ln0_b"])], axis=1)

    wout = np.zeros((256, DM), np.float32)
    wout[:D] = p["out_w"][:D]           # even-frame rows (j = d)
    wout[128:128 + D] = p["out_w"][D:]  # odd-frame rows
    sh["wout"] = _chunkB(wout)
    bout = np.zeros((256,), np.float32)
    bout[:D] = p["out_b"][:D]
    bout[128:128 + D] = p["out_b"][D:]
    sh["bout"] = _pp(bout)

    pos = _posenc_np(T, DM)  # (T, DM)
    sh["posT"] = np.ascontiguousarray(pos.T).reshape(NKC, 128, T)

    causal = np.zeros((4, 128, 256), np.float32)
    for kc, q0 in [(0, 0), (1, 0), (2, 256), (3, 256)]:
        k_idx = kc * 128 + np.arange(128)[:, None]
        q_idx = q0 + np.arange(256)[None, :]
        causal[kc] = (q_idx >= k_idx).astype(np.float32)
    sh["causal"] = causal
    sh["onesm"] = np.ones((128, 128), np.float32)
    sh["epsv"] = np.full((128, 1), 1e-6, np.float32)

    # per-core activations
    tf = np.zeros((B, D, T * RF), np.float32)
    tf[:, :, RF:] = target[:, :, :T * RF - RF]
    tgtT = np.zeros((B, 2, 128, T), np.float32)
    tgtT[:, 0, 0:D] = tf[:, :, 0::2]
    tgtT[:, 0, D:128] = tf[:, 0:48, 1::2]
    tgtT[:, 1, 0:32] = tf[:, 48:D, 1::2]

    kvbias = np.zeros((B, 128, NT), np.float32)
    k_global = np.arange(NT)[None, :] * 128 + np.arange(128)[:, None]
    for gb in range(B):
        kvbias[gb] = np.where(2 * k_global < data_len[gb], 0.0, NEG)

    per_core = []
    for c in range(NCORES):
        sl = slice(c * BL, (c + 1) * BL)
        per_core.append({
            "tgtT": np.ascontiguousarray(tgtT[sl]),
            "encT": np.ascontiguousarray(enc_output[sl]).reshape(BL, NKC, 128, T),
            "kvbias": np.ascontiguousarray(kvbias[sl]),
        })
    return sh, per_core


_SHARED_NAMES = {"pw1", "pw2", "pb", "ln0", "wq", "wk", "wv", "wsfc", "wcq",
                 "wck", "wcv", "wcfc", "w1", "w2", "b1", "b2", "lnp", "wout",
                 "bout", "posT", "causal", "onesm", "epsv"}


# ------------------------------------------------------------- runner
class _Runner:
    def __init__(self, nc):
        bass2jax.install_neuronx_cc_hook()
        self.nc = nc
        partition_name = nc.partition_id_tensor.name if nc.partition_id_tensor else None
        in_names, out_names, out_avals = [], [], []
        for alloc in nc.m.functions[0].allocations:
            if not isinstance(alloc, mybir.MemoryLocationSet):
                continue
            name = alloc.memorylocations[0].name
            if alloc.kind == "ExternalInput":
                if name != partition_name:
                    in_names.append(name)
            elif alloc.kind == "ExternalOutput":
                out_names.append(name)
                out_avals.append(jax.core.ShapedArray(
                    tuple(alloc.tensor_shape), mybir.dt.np(alloc.dtype)))
        self.in_names, self.out_names, self.out_avals = in_names, out_names, out_avals
        all_in = list(in_names) + list(out_names)
        if partition_name:
            all_in.append(partition_name)

        def _body(*args):
            operands = list(args)
            if partition_name:
                operands.append(bass2jax.partition_id_tensor())
            outs = bass2jax._bass_exec_p.bind(
                *operands, out_avals=tuple(out_avals), in_names=tuple(all_in),
                out_names=tuple(out_names), lowering_input_output_aliases=(),
                sim_require_finite=False, sim_require_nnan=False, nc=nc)
            return tuple(outs)

        devices = jax.devices()[:NCORES]
        self.mesh = Mesh(np.asarray(devices), ("core",))
        self.shard = NamedSharding(self.mesh, PartitionSpec("core"))
        self.repl = NamedSharding(self.mesh, PartitionSpec())
        in_specs = tuple(
            PartitionSpec() if n in _SHARED_NAMES else PartitionSpec("core")
            for n in in_names) + (PartitionSpec("core"),) * len(out_names)
        self._fn = jax.jit(
            shard_map(_body, mesh=self.mesh, in_specs=in_specs,
                      out_specs=(PartitionSpec("core"),) * len(out_names),
                      check_rep=False),
            keep_unused=True)
        self._zeros = None

    def prep(self, shared, per_core):
        args = []
        for n in self.in_names:
            if n in _SHARED_NAMES:
                args.append(jax.device_put(shared[n], self.repl))
            else:
                cat = np.ascontiguousarray(
                    np.concatenate([pc[n] for pc in per_core], axis=0))
                args.append(jax.device_put(cat, self.shard))
        if self._zeros is None:
            self._zeros = [
                jax.device_put(np.zeros((NCORES * av.shape[0], *av.shape[1:]),
                                        av.dtype), self.shard)
                for av in self.out_avals]
        return args + self._zeros

    def run(self, args):
        outs = self._fn(*args)
        jax.block_until_ready(outs)
        return outs


_CACHE = {}


def _get_runner(n_layers=L, debug_taps=False):
    key = (n_layers, debug_taps)
    if key not in _CACHE:
        _CACHE[key] = _Runner(_build(n_layers, debug_taps))
    return _CACHE[key]


def kernel(enc_output, target, data_len, max_len, params, n_layers=L,
           debug_taps=False):
    r = _get_runner(n_layers, debug_taps)
    shared, per_core = _prep_inputs(enc_output, target, data_len, params, n_layers)
    args = r.prep(shared, per_core)
    outs = r.run(args)
    res = {n: np.asarray(outs[i]) for i, n in enumerate(r.out_names)}
    out = res["outT"].reshape(B, D, T * RF)
    if debug_taps:
        taps = res["taps"].reshape(NCORES, n_layers + 1, BL, NKC, 128, T)
        return out, taps
    return out
